# revision 24
# baseline (speedup 1.0000x reference)
# Trainium2 Bass kernel for nn_MEMORY_34986803593776 (scatter_memory), v3.
#
# Math (per sample b):
#   w        = softmax(ck @ mk^T)                             [M]
#   c0       = qa * sigmoid(mem0 @ Wc0 + bc0)                 [DQA]
#   gate     = sigmoid(c0 @ Wm1 + bm1)                        [M*DV]
#   memPre   = mem0 * gate                                    [M*DV]
#   erase    = sig(sig(c0@We+be) + sig(memPre@Wemv+bemv))     [DV]
#   zt       = sig((c0@Wz+bz) + (memPre@Wzmv+bzmv))           [DV]
#   add      = tanh(tanh(zt@Wza+bza) + tanh(memPre@Wamv+bamv))[DV]
#   new      = memPre*(1 - w[m]*erase[dv]) + w[m]*add[dv]     [M,DV]
#
# Sharding: pure data parallel over batch B=16384 across 8 cores (2048/core).
#
# v3 changes over v2 (v2: 330us HW):
#  - The memPre^T -> natural transpose moved off the PE onto the DMA XBAR
#    (dma_start_transpose, measured 9.7us per 2MB tile, fans out over all
#    DMA engines). This kills 64 PE transposes + the PSUM->SBUF drain
#    copies on DVE/Act per tile. The natural free layout becomes
#    (c, s, p): sample = s*128+pb, f = c*128+p.
#  - e/a signals transposed by the same XBAR from a stacked [128, TB] tile.
#  - The final combine add (out = mn + u) runs on the otherwise-idle
#    GpSimd engine (measured 15.6us/tile, fits the tile period), freeing
#    a full DVE pass.
#  - Weights/prologue inputs load on the Activation HWDGE queue, the memT
#    tile stream + output on the SP queue; memT loads split into 4 chunk
#    DMAs so the c0 GEMM starts on partial data.

import numpy as np
import ml_dtypes

B = 16384
M = 64
DV = 64
DK = 64
DQA = 128
F = M * DV  # 4096
N_CORES = 8
B_CORE = B // N_CORES  # 2048
TB = 256                # samples per tile
NC2 = F // 128          # 32 f-chunks

_BUILD_CACHE = {}


def _build(b_core, iters, with_bm1):
    """Build and compile the single-core Bass program."""
    import concourse.tile as tile
    import concourse.bacc as bacc
    import concourse.mybir as mybir
    from concourse import masks
    from contextlib import ExitStack

    f32 = mybir.dt.float32
    bf16 = mybir.dt.bfloat16
    Alu = mybir.AluOpType
    Act = mybir.ActivationFunctionType

    NT = b_core // TB
    assert b_core % TB == 0

    nc = bacc.Bacc("TRN2", target_bir_lowering=False, debug=False,
                   num_devices=N_CORES)

    # ---- DRAM tensors (host-prepped layouts) ----
    d_memT = nc.dram_tensor("memT", (NT * 128, NC2 * TB), bf16,
                            kind="ExternalInput")
    d_qaT = nc.dram_tensor("qaT", (DQA, NT * TB), bf16, kind="ExternalInput")
    d_ckT = nc.dram_tensor("ckT", (DK, NT * TB), bf16, kind="ExternalInput")
    d_wc0 = nc.dram_tensor("wc0", (128, NC2 * 128), bf16, kind="ExternalInput")
    d_wm1 = nc.dram_tensor("wm1", (128, F), bf16, kind="ExternalInput")
    d_wez = nc.dram_tensor("wez", (128, NC2 * 128), bf16, kind="ExternalInput")
    d_wamv = nc.dram_tensor("wamv", (128, NC2 * 64), bf16, kind="ExternalInput")
    d_wewz = nc.dram_tensor("wewz", (128, 128), bf16, kind="ExternalInput")
    d_wza = nc.dram_tensor("wza", (DV, DV), bf16, kind="ExternalInput")
    d_mkt = nc.dram_tensor("mkt", (DK, M), bf16, kind="ExternalInput")
    d_bias = nc.dram_tensor("biasv", (128, 8), f32, kind="ExternalInput")
    if with_bm1:
        d_bm1 = nc.dram_tensor("bm1r", (1, F), bf16, kind="ExternalInput")
    # out[t, pb, (c, s, p)]: sample = t*TB + s*128 + pb, f = c*128 + p
    d_out = nc.dram_tensor("out", (NT * 128, 2 * F), bf16,
                           kind="ExternalOutput")

    memT_r = d_memT.ap().rearrange("(t p) f -> t p f", p=128)
    out_r = d_out.ap().rearrange("(t p) f -> t p f", p=128)

    with tile.TileContext(nc) as tc:
        with ExitStack() as ctx:
            wpool = ctx.enter_context(tc.tile_pool(name="wpool", bufs=1))
            pmem = ctx.enter_context(tc.tile_pool(name="pmem", bufs=2))
            pmpt = ctx.enter_context(tc.tile_pool(name="pmpt", bufs=2))
            pnat = ctx.enter_context(tc.tile_pool(name="pnat", bufs=2))
            pscr = ctx.enter_context(tc.tile_pool(name="pscr", bufs=2))
            pgate = ctx.enter_context(tc.tile_pool(name="pgate", bufs=2))
            sml = ctx.enter_context(tc.tile_pool(name="sml", bufs=2))
            pro = ctx.enter_context(tc.tile_pool(name="pro", bufs=1))
            ps_gate = ctx.enter_context(tc.tile_pool(name="ps_gate", bufs=3,
                                                     space="PSUM"))
            ps_sml = ctx.enter_context(tc.tile_pool(name="ps_sml", bufs=2,
                                                    space="PSUM"))

            # ---- tile-0-critical loads first on the Act queue: wc0 (split
            # so the c0 GEMM starts on the first half), then wm1, then the
            # small tensors (biases/qa needed ~12us in, prologue inputs
            # later still) ----
            w_c0 = wpool.tile([128, NC2, 128], bf16, tag="w_c0")
            wc0_src = d_wc0.ap().rearrange("k (c q) -> k c q", c=NC2)
            nc.scalar.dma_start(w_c0[:, 0:16, :], wc0_src[:, 0:16, :])
            nc.scalar.dma_start(w_c0[:, 16:32, :], wc0_src[:, 16:32, :])
            w_m1 = wpool.tile([128, NC2, 128], bf16, tag="w_m1")
            nc.scalar.dma_start(w_m1[:], d_wm1.ap().rearrange(
                "k (c q) -> k c q", c=NC2))
            biasv = wpool.tile([128, 8], f32, tag="biasv")
            nc.scalar.dma_start(biasv[:], d_bias.ap())
            qa_all = wpool.tile([128, NT, TB], bf16, tag="qa_all")
            nc.scalar.dma_start(qa_all[:], d_qaT.ap().rearrange(
                "q (t b) -> q t b", t=NT))
            w_mkt = wpool.tile([DK, M], bf16, tag="w_mkt")
            nc.scalar.dma_start(w_mkt[:], d_mkt.ap())
            ck_all = wpool.tile([64, NT, 2, 128], bf16, tag="ck_all")
            nc.scalar.dma_start(ck_all[:], d_ckT.ap().rearrange(
                "k (t s b) -> k t s b", t=NT, s=2))
            w_ewz = wpool.tile([128, 128], bf16, tag="w_ewz")
            nc.scalar.dma_start(w_ewz[:], d_wewz.ap())
            w_za = wpool.tile([DV, DV], bf16, tag="w_za")
            nc.scalar.dma_start(w_za[:], d_wza.ap())
            # wez/wamv (1.5MB, first needed ~30us in) load later — triggered
            # from whole() after tile 0's frontend so they don't contend
            # with memT(0)/wc0/wm1 on the ring at startup.
            w_ez = wpool.tile([128, NC2, 128], bf16, tag="w_ez")
            w_amv = wpool.tile([128, NC2, 64], bf16, tag="w_amv")

            def load_heavy_weights():
                nc.scalar.dma_start(w_ez[:], d_wez.ap().rearrange(
                    "k (c q) -> k c q", c=NC2))
                nc.scalar.dma_start(w_amv[:], d_wamv.ap().rearrange(
                    "k (c q) -> k c q", c=NC2))
            if with_bm1:
                bm1r = wpool.tile([1, F], bf16, tag="bm1r")
                nc.scalar.dma_start(bm1r[:], d_bm1.ap())
                ones_b = wpool.tile([1, TB], bf16, tag="ones_b")
                nc.vector.memset(ones_b[:], 1.0)
            ident = wpool.tile([128, 128], bf16, tag="ident")
            masks.make_identity(nc, ident[:])

            bc0 = biasv[:, 0:1]
            b_e = biasv[0:64, 1:2]
            b_emv = biasv[0:64, 3:4]
            b_zmv = biasv[0:64, 4:5]
            b_amv = biasv[0:64, 5:6]
            b_za = biasv[0:64, 6:7]

            def prologue(w_nat_all):
                """w = softmax(ck @ mk^T) for all tiles, natural [b, m].

                Batched: 8 (t, s) pairs share one PSUM bank + one exp pass.
                """
                w_flat = w_nat_all[:].rearrange("p t s d -> p (t s) d")
                ckf = ck_all[:].rearrange("k t s b -> k (t s) b")
                for j0 in range(0, 2 * NT, 8):
                    n = min(8, 2 * NT - j0)
                    lgt = ps_gate.tile([128, 4, TB], f32, tag="gate")
                    lgv = lgt[:].rearrange("p c b -> p (c b)")[:, 0:512] \
                        .rearrange("p (j d) -> p j d", d=M)
                    for j in range(n):
                        nc.tensor.matmul(lgv[:, j], ckf[:, j0 + j, :],
                                         w_mkt[:], start=True, stop=True)
                    exv = sml.tile([128, 8, M], f32, tag="exv")
                    nc.scalar.activation(exv[:, 0:n], lgv[:, 0:n], Act.Exp)
                    sm = sml.tile([128, 8, 1], f32, tag="sm")
                    nc.vector.tensor_reduce(sm[:, 0:n], exv[:, 0:n],
                                            mybir.AxisListType.X, Alu.add)
                    nc.vector.reciprocal(sm[:, 0:n], sm[:, 0:n])
                    nc.vector.tensor_tensor(
                        w_flat[:, j0:j0 + n, :], exv[:, 0:n],
                        sm[:, 0:n].broadcast_to([128, n, M]), op=Alu.mult)

            def load_tile(t):
                mt = pmem.tile([128, NC2 * TB], bf16, tag="memT")
                mtv = mt[:].rearrange("p (q x) -> p q x", q=4)
                src = memT_r[t].rearrange("p (q x) -> p q x", q=4)
                for q in range(4):
                    nc.sync.dma_start(mtv[:, q], src[:, q])
                return mt

            def frontendA(t, mt):
                """c0 = qa * sigmoid(mem @ Wc0 + bc0), T layout.

                The qa multiply runs on GpSimd: it's tiny (256 free), and on
                DVE it would sit after the combine in queue order, making
                the next tile's gate GEMMs stall ~7us for it."""
                mtv = mt[:].rearrange("p (c b) -> p c b", b=TB)
                c0t = ps_gate.tile([128, 4, TB], f32, tag="gate")
                c0ps = c0t[:, 0, :]
                for c in range(NC2):
                    nc.tensor.matmul(c0ps, w_c0[:, c, :], mtv[:, c, :],
                                     start=(c == 0), stop=(c == NC2 - 1))
                c0s = sml.tile([128, TB], bf16, tag="c0s")
                nc.scalar.activation(c0s[:], c0ps, Act.Sigmoid, bias=bc0)
                c0qa = sml.tile([128, TB], bf16, tag="c0qa")
                nc.vector.tensor_tensor(c0qa[:], c0s[:], qa_all[:, t, :],
                                        op=Alu.mult)
                return dict(mt=mt, c0qa=c0qa)

            def gate_mpre(st, fin_st=None):
                """gate -> mpreT for st's tile. If fin_st is given, its
                ea_fin is interleaved after group 2 so ea_nat lands well
                before the next combine instead of queueing behind all
                eight gate sigmoids."""
                mt, c0qa = st["mt"], st["c0qa"]
                mpt = pmpt.tile([128, NC2 * TB], bf16, tag="mpreT")
                gsb = None
                for g in range(8):           # 8 groups of 4 chunks
                    if g % 2 == 0:
                        gsb = pgate.tile([128, 8, TB], bf16, tag="gateT")
                    pg = ps_gate.tile([128, 4, TB], f32, tag="gate")
                    for j in range(4):
                        c = g * 4 + j
                        nc.tensor.matmul(pg[:, j], w_m1[:, c, :], c0qa[:],
                                         start=True, stop=not with_bm1)
                        if with_bm1:
                            nc.tensor.matmul(pg[:, j],
                                             bm1r[:, c * 128:(c + 1) * 128],
                                             ones_b[:], start=False, stop=True)
                    nc.scalar.activation(gsb[:, (g % 2) * 4:(g % 2) * 4 + 4, :],
                                         pg[:], Act.Sigmoid)
                    if g % 2 == 1:
                        half = slice((g - 1) * 4 * TB, (g + 1) * 4 * TB)
                        nc.vector.tensor_tensor(
                            mpt[:, half],
                            mt[:, half],
                            gsb[:].rearrange("p c b -> p (c b)"),
                            op=Alu.mult)
                    if g == 2 and fin_st is not None:
                        ea_fin(fin_st)
                st["mpt"] = mpt
                return st

            def alloc_mv(st):
                """PSUM accumulators for the mv GEMMs of st's tile."""
                ezt = ps_sml.tile([128, 2, TB], f32, tag="ez", bufs=1)
                st["ez"] = ezt[:, 0, :]
                st["av"] = ezt[0:64, 1, :]
                st["za"] = ezt[64:128, 1, :]
                return st

            def backend_pre(t, st):
                """Epilogue head for tile t: wz + mv GEMMs (PE), the first
                activations, and the mn XBAR. No DVE ops here — DVE's next
                queued work must be the previous tile's combine."""
                mpt, c0qa = st["mpt"], st["c0qa"]
                ez, av = st["ez"], st["av"]
                pv = mpt[:].rearrange("p (c b) -> p c b", b=TB)

                # PE: wz first (needs only c0qa), then the mv GEMMs
                wzt = ps_gate.tile([128, 4, TB], f32, tag="gate")
                wz = wzt[:, 0, :]
                nc.tensor.matmul(wz, w_ewz[:], c0qa[:], start=True, stop=True)
                for c in range(NC2):
                    nc.tensor.matmul(ez, w_ez[:, c, :], pv[:, c, :],
                                     start=(c == 0), stop=(c == NC2 - 1))
                for c in range(NC2):
                    nc.tensor.matmul(av, w_amv[:, c, :], pv[:, c, :],
                                     start=(c == 0), stop=(c == NC2 - 1))

                # ---- mpreT -> natural via the DMA XBAR ----
                # mn[pb, (c, s, p)] = mpre[sample (s,pb)][f = c*128+p]
                # On the SP queue: HWDGE queues retire in order, and a
                # DMA_TRANSPOSE holds its queue ~10us — on the Act queue it
                # would stall every activation behind it.
                mn = pnat.tile([128, 2 * F], bf16, tag="mn")
                nc.sync.dma_start(
                    mn[:].rearrange("p (k i) -> p k i", i=128), mpt[:],
                    transpose=True)

                # Act chain heads (issue order = readiness order)
                ecT = sml.tile([64, TB], bf16, tag="ecT")
                nc.scalar.activation(ecT[:], wz[0:64], Act.Sigmoid, bias=b_e)
                emvT = sml.tile([64, TB], bf16, tag="emvT")
                nc.scalar.activation(emvT[:], ez[0:64], Act.Sigmoid, bias=b_emv)
                # b_zmv column holds bzmv + bz (host-folded), so the zt
                # chain needs no separate bias pass for Wz's output.
                zmvT = sml.tile([64, TB], bf16, tag="zmvT")
                nc.scalar.activation(zmvT[:], ez[64:128], Act.Identity,
                                     bias=b_zmv)
                amvT = sml.tile([64, TB], bf16, tag="amvT")
                nc.scalar.activation(amvT[:], av, Act.Tanh, bias=b_amv)

                st["wz"] = wz
                st["ecT"] = ecT
                st["emvT"] = emvT
                st["zmvT"] = zmvT
                st["amvT"] = amvT
                st["mn"] = mn

            def backend_mid(t, w_nat_all, prev, st):
                """Combine of tile t-1 (prev) interleaved with tile t's (st)
                small epilogue chain, in DVE-readiness order."""
                if prev is not None:
                    mn, ea_nat, w2 = prev["mn"], prev["ea_nat"], prev["w2"]
                    u = pscr.tile([128, 2 * F], bf16, tag="u")
                    mnv = mn[:].rearrange("p (c s h d) -> p c s h d",
                                          c=NC2, s=2, h=2)
                    uv = u[:].rearrange("p (c s h d) -> p c s h d",
                                        c=NC2, s=2, h=2)
                    ugv = u[:].rearrange("p (c s h g r) -> p c s h g r",
                                         c=NC2, s=2, h=2, r=2)
                    w2v = w2[:].rearrange("p s (c h r) -> p c s h r",
                                          c=NC2, h=2, r=2)

                def combine_half(s):
                    ebc = ea_nat[:, s, 0:64].unsqueeze(1).unsqueeze(2) \
                        .broadcast_to([128, NC2, 2, 64])
                    abc = ea_nat[:, s, 64:128].unsqueeze(1).unsqueeze(2) \
                        .broadcast_to([128, NC2, 2, 64])
                    nc.vector.tensor_tensor(uv[:, :, s], mnv[:, :, s], ebc,
                                            op=Alu.mult)
                    nc.vector.tensor_tensor(uv[:, :, s], abc, uv[:, :, s],
                                            op=Alu.subtract)
                    for h in range(2):
                        wb = w2v[:, :, s, h].unsqueeze(2) \
                            .broadcast_to([128, NC2, 32, 2])
                        nc.vector.tensor_tensor(ugv[:, :, s, h],
                                                ugv[:, :, s, h], wb,
                                                op=Alu.mult)

                if prev is not None:
                    combine_half(0)

                if st is not None:
                    wz = st["wz"]
                    esum = sml.tile([64, TB], bf16, tag="esum")
                    nc.vector.tensor_tensor(esum[:], st["ecT"][:],
                                            st["emvT"][:], op=Alu.add)
                    zsum = sml.tile([64, TB], bf16, tag="zc")
                    nc.vector.tensor_tensor(zsum[:], st["zmvT"][:],
                                            wz[64:128], op=Alu.add)
                    ztT = sml.tile([64, TB], bf16, tag="ztT")
                    nc.scalar.activation(ztT[:], zsum[:], Act.Sigmoid)
                    nc.tensor.matmul(st["za"], w_za[:], ztT[:], start=True,
                                     stop=True)

                if prev is not None:
                    combine_half(1)

                if st is not None:
                    zaT = sml.tile([64, TB], bf16, tag="zaT")
                    nc.scalar.activation(zaT[:], st["za"], Act.Tanh, bias=b_za)
                    asum = sml.tile([64, TB], bf16, tag="asum")
                    nc.vector.tensor_tensor(asum[:], zaT[:], st["amvT"][:],
                                            op=Alu.add)
                    # e (sigmoid) into top, a (tanh) into bottom of a stacked
                    # [128, TB] tile; PE-transposed to ea_nat after the next
                    # gate GEMMs (see ea_fin).
                    east = sml.tile([128, TB], bf16, tag="east")
                    nc.scalar.activation(east[0:64, :], esum[:], Act.Sigmoid)
                    nc.scalar.activation(east[64:128, :], asum[:], Act.Tanh)
                    st["east"] = east

                if prev is not None:
                    # final add on DVE. (GpSimd runs at model speed, but a
                    # concurrent DVE TT crawls ~6x while GpSimd holds the
                    # shared SBUF ports — a large net loss.) The drain tile
                    # splits add+store by s-half so the output DMA starts
                    # ~2us earlier.
                    if st is None:
                        mnv2 = mn[:].rearrange("p (c s x) -> p c s x",
                                               c=NC2, s=2)
                        uv2 = u[:].rearrange("p (c s x) -> p c s x",
                                             c=NC2, s=2)
                        outv = out_r[t - 1].rearrange(
                            "p (c s2 x) -> p c s2 x", c=NC2, s2=2)
                        for s in range(2):
                            nc.vector.tensor_tensor(mnv2[:, :, s],
                                                    mnv2[:, :, s],
                                                    uv2[:, :, s], op=Alu.add)
                            nc.sync.dma_start(outv[:, :, s], mnv2[:, :, s])
                    else:
                        nc.vector.tensor_tensor(mn[:], mn[:], u[:],
                                                op=Alu.add)
                        nc.sync.dma_start(out_r[t - 1], mn[:])

                if st is not None:
                    # pair-duplicated w: combine innermost AP step stays 1.
                    # On Act (copy engine-wise) to keep it off the DVE queue.
                    w2 = sml.tile([128, 2, 128], bf16, tag="w2")
                    nc.scalar.copy(
                        w2[:].rearrange("p s (m r) -> p s m r", r=2),
                        w_nat_all[:, t, :, :].unsqueeze(3)
                        .broadcast_to([128, 2, M, 2]))
                    st["w2"] = w2

            def ea_fin(st):
                """east -> ea_nat[pb, s, which*64+dv] via two PE transposes
                + one Act copy. Emitted after the next tile's gate GEMMs so
                the PE queue head never waits on east."""
                east = st["east"]
                eatp = ps_sml.tile([128, 2, 128], bf16, tag="eatp", bufs=1)
                for hh in range(2):
                    nc.tensor.transpose(eatp[:, hh, :],
                                        east[:, hh * 128:(hh + 1) * 128],
                                        ident[:])
                ea_nat = sml.tile([128, 2, 128], bf16, tag="eanat")
                nc.scalar.copy(ea_nat[:], eatp[:])
                st["ea_nat"] = ea_nat

            def whole():
                # Software-pipelined: tile t's combine runs one iteration
                # late, overlapping the 10us mn XBAR(t) with the previous
                # tile's DVE combine; the small epilogue chain of tile t is
                # interleaved into the combine stream at readiness points.
                w_nat_all = pro.tile([128, NT, 2, M], bf16, tag="w_nat_all")
                mt0 = load_tile(0)
                st = frontendA(0, mt0)
                load_heavy_weights()
                st = gate_mpre(st)
                # prologue AFTER tile 0's frontend/gate: its exp table load
                # and DVE ops would otherwise head-block tile 0's chain on
                # the Act/DVE queues; w_nat is first needed by w2(0) an
                # entire iteration later.
                prologue(w_nat_all)
                st = alloc_mv(st)
                prev = None
                for t in range(NT):
                    st_next = None
                    if t + 1 < NT:
                        mt_next = load_tile(t + 1)
                    backend_pre(t, st)
                    backend_mid(t, w_nat_all, prev, st)
                    if t + 1 < NT:
                        st_next = frontendA(t + 1, mt_next)
                        st_next = gate_mpre(st_next, fin_st=st)
                        st_next = alloc_mv(st_next)
                    else:
                        ea_fin(st)
                    prev = st
                    st = st_next
                backend_mid(NT, w_nat_all, prev, None)

            if iters == 1:
                whole()
            else:
                with tc.For_i(0, iters, 1,
                              hint_engines=(mybir.EngineType.PE,
                                            mybir.EngineType.DVE,
                                            mybir.EngineType.Activation,
                                            mybir.EngineType.Pool,
                                            mybir.EngineType.SP)):
                    whole()

    nc.compile()
    return nc


def _get_nc(b_core, iters, with_bm1):
    key = (b_core, iters, with_bm1)
    if key not in _BUILD_CACHE:
        _BUILD_CACHE[key] = _build(b_core, iters, with_bm1)
    return _BUILD_CACHE[key]


def _prep_weights(inputs):
    bf = ml_dtypes.bfloat16
    wc0 = np.ascontiguousarray(
        inputs["Wc0"].reshape(NC2, 128, 128).transpose(1, 0, 2).reshape(128, -1)
    ).astype(bf)
    # Wm1 [128 q, 4096 f]: stationary chunks [q, f-chunk] — natural layout
    wm1 = np.ascontiguousarray(inputs["Wm1"]).astype(bf)
    wez_full = np.concatenate([inputs["Wemv"], inputs["Wzmv"]], axis=1)
    wez = np.ascontiguousarray(
        wez_full.reshape(NC2, 128, 128).transpose(1, 0, 2).reshape(128, -1)
    ).astype(bf)
    wamv = np.ascontiguousarray(
        inputs["Wamv"].reshape(NC2, 128, 64).transpose(1, 0, 2).reshape(128, -1)
    ).astype(bf)
    wewz = np.concatenate([inputs["We"], inputs["Wz"]], axis=1).astype(bf)
    wza = inputs["Wza"].astype(bf)
    mkt = np.ascontiguousarray(inputs["memory_key"].T).astype(bf)

    biasv = np.zeros((128, 8), np.float32)
    biasv[:, 0] = inputs["bc0"]
    biasv[0:64, 1] = inputs["be"]
    biasv[0:64, 2] = inputs["bz"]
    biasv[0:64, 3] = inputs["bemv"]
    # bz folded into the zmv bias (the zt chain adds wz raw from PSUM)
    biasv[0:64, 4] = inputs["bzmv"] + inputs["bz"]
    biasv[0:64, 5] = inputs["bamv"]
    biasv[0:64, 6] = inputs["bza"]

    w = dict(wc0=wc0, wm1=wm1, wez=wez, wamv=wamv, wewz=wewz, wza=wza,
             mkt=mkt, biasv=biasv)
    with_bm1 = bool(np.any(inputs["bm1"]))
    if with_bm1:
        w["bm1r"] = inputs["bm1"].reshape(1, F).astype(bf)
    return w, with_bm1


def _make_in_maps(inputs, b_core):
    bf = ml_dtypes.bfloat16
    NT = b_core // TB
    wdict, _ = _prep_weights(inputs)
    mem = np.asarray(inputs["memory_value"], np.float32).reshape(B, F)
    qa = np.asarray(inputs["control_qa"], np.float32)
    ck = np.asarray(inputs["control_key"], np.float32)
    in_maps = []
    for core in range(N_CORES):
        sl = slice(core * b_core, (core + 1) * b_core)
        # memT[t, p, c, b=(s,pb)] = mem[t*TB + s*128 + pb, c*128 + p]
        m = mem[sl].reshape(NT, 2, 128, NC2, 128)          # t s pb c p
        m = np.ascontiguousarray(m.transpose(0, 4, 3, 1, 2))  # t p c s pb
        memT = m.reshape(NT * 128, NC2 * TB).astype(bf)
        q = qa[sl].reshape(NT, TB, DQA).transpose(2, 0, 1)   # q t b
        qaT = np.ascontiguousarray(q).reshape(DQA, NT * TB).astype(bf)
        c = ck[sl].reshape(NT, TB, DK).transpose(2, 0, 1)
        ckT = np.ascontiguousarray(c).reshape(DK, NT * TB).astype(bf)
        in_maps.append(dict(memT=memT, qaT=qaT, ckT=ckT, **wdict))
    return in_maps


def kernel(**inputs):
    from concourse import bass_utils
    inputs = {k: np.asarray(v) for k, v in inputs.items()}
    _, with_bm1 = _prep_weights(inputs)
    nc = _get_nc(B_CORE, 1, with_bm1)
    in_maps = _make_in_maps(inputs, B_CORE)
    res = bass_utils.run_bass_kernel_spmd(nc, in_maps,
                                          core_ids=list(range(N_CORES)))
    NT = B_CORE // TB
    outs = []
    for r in res.results:
        # out[t, pb, c, s, p]: sample = t*TB + s*128 + pb, f = c*128 + p
        o = np.asarray(r["out"], np.float32).reshape(NT, 128, NC2, 2, 128)
        o = o.transpose(0, 3, 1, 2, 4).reshape(B_CORE, F)  # t s pb (c p)
        outs.append(o)
    out = np.concatenate(outs, axis=0)
    return out.reshape(B, M, DV).astype(np.float32)
